# revision 23
# baseline (speedup 1.0000x reference)
"""Trainium2 Bass kernel for LocalSelfAttention (sliding-window attention).

Reference computation (fp32):
  qkv = x @ W_qkv ; q /= 8 ; sliding window of 7 keys (3 each side, zero-padded)
  attn = softmax(q . k_win + pos_bias) ; out = (attn @ v_win) @ W_out

Sharding: data-parallel over B*HW = 128 independent rows -> 16 rows per core.
Each core processes its rows in 8 pairs (512 tokens per pair).

Per-core layout, software-pipelined 5 pairs deep so the PE never stalls on
the vector-engine softmax chain:
  stage A1(p): xT arrives pre-transposed from host (bf16 + fp8 copies,
               plain DMA); qkT = W_qk^T. @ xT in fp8 DoubleRow;
               V = xT^T. @ W_v (bf16)
  stage A2(p): scores ST[key,q] per head; exp w/ folded 1/sqrt(dk) and fp8
               descale (Scalar); * expB band mask (Pool)
  stage B1(p): denom = ones^T. @ attn_un (replicated across 64 partitions,
               2 heads per psum tile) + rank-2 matmul U^T. @ E adding the
               zero-pad edge correction; reciprocal_approx_fast from PSUM
  stage B2(p): avT = V^T. @ attn_un; * recip while copying PSUM->SBUF
  stage C(p):  out = avT^T. @ W_out -> DMA

Steady-state emission per step t: A1(t), then scores(t-1) interleaved
head-by-head with den(t-2)/av(t-3) groups (covers PSUM-rotation waits with
PE work), then out-proj C(t-4).
"""

import numpy as np
import ml_dtypes

import concourse.bass as bass
import concourse.tile as tile
from concourse import bacc, mybir
from concourse.bass_utils import run_bass_kernel_spmd

# Problem constants (hardcoded per contract)
B, HW, S, D = 2, 64, 256, 512
HEADS, DK, KSIZE, PAD = 8, 64, 7, 3
HDK = HEADS * DK            # 512
QK = 2 * HDK                # 1024 (q and k dims)
N_CORES = 8
ROWS_PER_CORE = (B * HW) // N_CORES   # 16
PAIRS = ROWS_PER_CORE // 2            # 8
PTOK = 2 * S                          # 512 tokens per pair
P = 128
NCH = S // P                          # 2 key chunks per row
STRIPE = 132                          # query stripe width per key chunk (even)
STRIPE_PAD = 256                      # psum slot per (chunk,row) stripe, bank aligned
STRIPE_START = (0, S - STRIPE)        # stripe start per chunk within a row
HPAIRS = HEADS // 2                   # 4 head pairs packed on 64+64 partitions
KO_G = D // 128                       # 4 k-chunks of x along D

F32 = mybir.dt.float32
BF16 = mybir.dt.bfloat16
FP8 = mybir.dt.float8e4
FP8_WSCALE = 32.0                     # fp8 qk weight pre-scale (power of 2)
EXP_SCALE = 1.0 / (FP8_WSCALE * FP8_WSCALE * 8.0)  # undo w-scales + 1/sqrt(DK)

_CACHE = {}


def _host_constants(pos_bias, W_qkv, W_out):
    """Host-precomputed tensors: fp8 qk weights, bf16 v/out weights, expB
    band mask, rank-2 edge-correction factors."""
    W1 = W_qkv.astype(np.float32).copy()
    # qk weights scaled up by 32 to sit in fp8e4m3's normal range (std 0.02
    # would straddle the 2^-6 min normal); compensated in the exp scale,
    # which also folds in the 1/sqrt(DK) query scaling.
    W1qk = (W1[:, :QK] * FP8_WSCALE).astype(ml_dtypes.float8_e4m3)  # [512, 1024]
    W1v = W1[:, QK:].astype(ml_dtypes.bfloat16)               # [512, 512]
    W2 = W_out.astype(np.float32).astype(ml_dtypes.bfloat16)  # [512, 512]

    pb = pos_bias.astype(np.float32)              # [H, S, KSIZE]
    # expB[j, h, c, q'] : key j (within chunk c), query q = STRIPE_START[c] + q'
    # value exp(pos_bias[h, q, w]) with w = (j_global - q) + PAD if in band else 0
    j = np.arange(P)[:, None, None, None]
    h = np.arange(HEADS)[None, :, None, None]
    c = np.arange(NCH)[None, None, :, None]
    qp = np.arange(STRIPE)[None, None, None, :]
    q_glob = np.array(STRIPE_START)[None, None, :, None] + qp
    j_glob = c * P + j
    w = j_glob - q_glob + PAD
    in_band = (w >= 0) & (w < KSIZE)
    w_c = np.clip(w, 0, KSIZE - 1)
    bias_val = pb[h, q_glob, w_c]
    expB = np.where(in_band, np.exp(bias_val), 0.0).astype(np.float32)
    expB = expB.astype(ml_dtypes.bfloat16)        # [128, H, NCH, STRIPE]

    # edge correction: sum over out-of-range window slots of exp(bias).
    # Shipped as the moving operand E of a rank-2 matmul U^T. @ E that
    # accumulates it straight into the denominator PSUM: row i of E is the
    # correction for head 2j+i, row i of U selects partition half i.
    q = np.arange(S)[None, :, None]
    w2 = np.arange(KSIZE)[None, None, :]
    oor = ((q + w2 - PAD) < 0) | ((q + w2 - PAD) >= S)
    ec = (np.exp(pb) * oor).sum(-1)               # [H, S]
    # only q in {0,1,2} and {S-3..S-1} have nonzero correction; ship just
    # those 12 columns per row pair: ecE[i, j, r, s, e] = ec[2j+i, s*253+e]
    ecE = np.empty((2, HPAIRS, 2, 2, 3), np.float32)
    for jj in range(HPAIRS):
        for i in range(2):
            for s in range(2):
                ecE[i, jj, :, s, :] = ec[2 * jj + i, s * 253:s * 253 + 3][None]
    return W1qk, W1v, W2, expB, ecE.astype(ml_dtypes.bfloat16)


def _build_nc():
    nc = bacc.Bacc(None, target_bir_lowering=False)
    # x shipped pre-transposed (and pre-cast to fp8 for the qk path) from the
    # host: xT[p, ko, t] = x[t, ko*128+p]. Plain DMAs, no dma_transpose, no
    # on-device fp8 cast.
    xt_d = nc.dram_tensor("xt", [P, KO_G, ROWS_PER_CORE * S], BF16,
                          kind="ExternalInput")
    x8_d = nc.dram_tensor("x8", [P, KO_G, ROWS_PER_CORE * S], FP8,
                          kind="ExternalInput")
    w1qk_d = nc.dram_tensor("w1qk", [D, QK], FP8, kind="ExternalInput")
    w1v_d = nc.dram_tensor("w1v", [D, HDK], BF16, kind="ExternalInput")
    w2_d = nc.dram_tensor("w2", [HDK, D], BF16, kind="ExternalInput")
    expb_d = nc.dram_tensor("expb", [P, HEADS, NCH, STRIPE], BF16, kind="ExternalInput")
    ec_d = nc.dram_tensor("ec", [2, HPAIRS, 2, 2, 3], BF16, kind="ExternalInput")
    u_d = nc.dram_tensor("u", [2, P], BF16, kind="ExternalInput")
    out_d = nc.dram_tensor("out", [ROWS_PER_CORE * S, D], BF16, kind="ExternalOutput")

    KO = D // P      # 4 K-chunks for projections
    TC = PTOK // P   # 4 token chunks per pair
    QKC = QK // P    # 8 qk output chunks
    HC = HDK // P    # 4 hdk chunks
    DR = mybir.MatmulPerfMode.DoubleRow
    COPY = mybir.ActivationFunctionType.Copy

    with tile.TileContext(nc) as tc:
        with (
            tc.tile_pool(name="const", bufs=1) as const,
            tc.tile_pool(name="io", bufs=3) as io,
            tc.tile_pool(name="early", bufs=2) as early,
            tc.tile_pool(name="vpool", bufs=3) as vpool,
            tc.tile_pool(name="attn", bufs=3) as attnp,
            tc.tile_pool(name="bpool", bufs=2) as bpool,
            tc.tile_pool(name="ps_proj", bufs=4, space="PSUM") as ps_proj,
            tc.tile_pool(name="ps_st", bufs=2, space="PSUM") as ps_st,
        ):
            # ---- constants; first x transpose goes ahead of the fat consts
            # (everything rides the sync queue, scalar stays free for copies)
            w1qk_sb = const.tile([P, KO, QK], FP8)
            w1v_sb = const.tile([P, KO, HDK], BF16)
            expb_sb = const.tile([P, HEADS, NCH, STRIPE], BF16)
            w2_sb = const.tile([P, HC, D], BF16)
            ecE_sb = const.tile([2, HPAIRS, 2, 2, 3], BF16)
            u_sb = const.tile([2, P], BF16)
            ones_sb = const.tile([P, 64], BF16)
            nc.vector.memset(ones_sb, 1.0)

            def load_w1qk():
                nc.sync.dma_start(
                    w1qk_sb[:], w1qk_d.rearrange("(ko ki) n -> ki ko n", ki=P))

            def load_w1v():
                nc.sync.dma_start(
                    w1v_sb[:], w1v_d.rearrange("(ko ki) n -> ki ko n", ki=P))

            warm = {}

            def warmup_pe():
                # dummy matmuls during the initial DMA wait: the PE needs ~3us
                # of continuous busy to reach its full 2.4GHz p-state
                scratch = const.tile([P, PTOK], BF16, name="warm_scr")
                nc.gpsimd.memset(scratch, 0.0)
                # rides the regular p512 rotation so no PSUM bank stays
                # reserved for warmup after the fill phase
                wps = ps_proj.tile([P, PTOK], F32, tag="p512", name="wps")
                warm["scratch"], warm["wps"] = scratch, wps
                for i in range(6):
                    nc.tensor.matmul(
                        wps[0:64, :], ones_sb[:], scratch[:],
                        start=True, stop=True,
                    )

            def load_consts_rest():
                nc.sync.dma_start(expb_sb[:], expb_d[:])
                nc.sync.dma_start(
                    w2_sb[:], w2_d.rearrange("(hc ki) n -> ki hc n", ki=P))
                nc.sync.dma_start(ecE_sb[:], ec_d[:])
                nc.sync.dma_start(u_sb[:], u_d[:])

            xT_tiles = {}
            x8_tiles = {}
            qkT_tiles = {}
            attn_tiles = {}
            recip_tiles = {}
            v_tiles = {}
            avT_tiles = {}

            def stage_load8(pr):
                # x8/xT ride separate DGE queues (gpsimd/vector issue) so the
                # cold start isn't serialized behind the const loads on sync
                x8 = io.tile([P, KO_G, PTOK], FP8, tag="x8")
                nc.sync.dma_start(
                    x8[:], x8_d[:, :, pr * PTOK:(pr + 1) * PTOK])
                x8_tiles[pr] = x8

            def stage_load(pr):
                xT = io.tile([P, KO_G, PTOK], BF16, tag="xT")
                nc.sync.dma_start(
                    xT[:], xt_d[:, :, pr * PTOK:(pr + 1) * PTOK])
                xT_tiles[pr] = xT

            def stage_a1(pr):
                xT = xT_tiles.pop(pr)
                xT8 = x8_tiles.pop(pr)

                # qk projection in fp8 DoubleRow: qkT [qk dims, tokens]
                qkT = early.tile([P, QKC, PTOK], BF16, tag="qkT", bufs=3)
                qkT_tiles[pr] = qkT
                for m in range(QKC):
                    pp = ps_proj.tile([P, PTOK], F32, tag="p512")
                    for kp in range(KO // 2):
                        nc.tensor.matmul(
                            pp[:],
                            w1qk_sb[:, 2 * kp:2 * kp + 2, m * P:(m + 1) * P],
                            xT8[:, 2 * kp:2 * kp + 2, :],
                            start=(kp == 0), stop=(kp == KO // 2 - 1),
                            perf_mode=DR,
                        )
                    if m % 2 == 0:
                        nc.scalar.activation(qkT[:, m, :], pp[:], func=COPY)
                    else:
                        nc.vector.tensor_copy(qkT[:, m, :], pp[:])

                # v projection (bf16): V [tokens, hdk]
                v_sb = vpool.tile([P, TC, HDK], BF16, tag="v_sb", bufs=4)
                v_tiles[pr] = v_sb
                for tcc in range(TC):
                    pp = ps_proj.tile([P, PTOK], F32, tag="p512")
                    for ko in range(KO):
                        nc.tensor.matmul(
                            pp[:],
                            xT[:, ko, tcc * P:(tcc + 1) * P],
                            w1v_sb[:, ko, :],
                            start=(ko == 0), stop=(ko == KO - 1),
                        )
                    if tcc % 2 == 0:
                        nc.scalar.activation(v_sb[:, tcc, :], pp[:], func=COPY)
                    else:
                        nc.vector.tensor_copy(v_sb[:, tcc, :], pp[:])

            def scores_head(pr, h, fast_mask=False):
                # one head's scores + exp; band mask batched per 4 heads on
                # gpsimd (a full pipeline stage of slack), except the last
                # pair where per-head DVE masks shorten the drain chain
                qkT = qkT_tiles[pr]
                mq = h // 2          # q chunk index in qkT
                mk = 4 + h // 2      # k chunk index in qkT
                p0 = 64 * (h % 2)    # partition offset within chunk
                sl = slice(p0, p0 + 64)

                st = ps_st.tile([P, NCH, 2, STRIPE_PAD], F32, tag="st")
                for c in range(NCH):
                    for r in range(2):
                        nc.tensor.matmul(
                            st[:, c, r, :STRIPE],
                            qkT[sl, mk, r * S + c * P:r * S + (c + 1) * P],
                            qkT[sl, mq,
                                r * S + STRIPE_START[c]:
                                r * S + STRIPE_START[c] + STRIPE],
                            start=True, stop=True,
                        )
                if h == 0:
                    attn_tiles[pr] = attnp.tile(
                        [P, HEADS, NCH, 2, STRIPE], BF16, tag="attn",
                        name="attn_m")
                attn_m = attn_tiles[pr]
                nc.scalar.activation(
                    attn_m[:, h], st[:, :, :, :STRIPE],
                    func=mybir.ActivationFunctionType.Exp,
                    scale=EXP_SCALE)
                if fast_mask:
                    nc.vector.tensor_tensor(
                        attn_m[:, h], attn_m[:, h],
                        expb_sb[:, h, :, None, :].to_broadcast(
                            (P, NCH, 2, STRIPE)),
                        mybir.AluOpType.mult,
                    )
                elif h % 4 == 3:
                    h0 = h - 3
                    nc.gpsimd.tensor_tensor(
                        attn_m[:, h0:h0 + 4], attn_m[:, h0:h0 + 4],
                        expb_sb[:, h0:h0 + 4, :, None, :].to_broadcast(
                            (P, 4, NCH, 2, STRIPE)),
                        mybir.AluOpType.mult,
                    )

            def den_group(pr, j):
                # denominators for head pair j: ones-matmuls + rank-2 edge
                # correction accumulated in PSUM, then approx reciprocal
                attn_m = attn_tiles[pr]
                if j == 0:
                    recip_tiles[pr] = bpool.tile(
                        [P, HPAIRS, PTOK], F32, tag="recip_rep",
                        name="recip_rep", bufs=3)
                recip_rep = recip_tiles[pr]
                den = ps_proj.tile([P, PTOK], F32, tag="p512")
                for h in (2 * j, 2 * j + 1):
                    p0 = 64 * (h % 2)
                    sl = slice(p0, p0 + 64)
                    tpos = None if p0 == 0 else (0, 64)
                    for r in range(2):
                        for c in range(NCH):
                            nc.tensor.matmul(
                                den[sl, r * S + STRIPE_START[c]:
                                        r * S + STRIPE_START[c] + STRIPE],
                                ones_sb[:],
                                attn_m[:, h, c, r, :],
                                start=(r == 0 and c == 0),
                                stop=False,
                                tile_position=tpos,
                            )
                base = den[:, 0:3]
                edge_ap = bass.AP(
                    base.tensor, base.offset,
                    [list(base.ap[0]), [256, 2], [253, 2], list(base.ap[1])])
                nc.tensor.matmul(
                    edge_ap, u_sb[:, :], ecE_sb[:, j],
                    start=False, stop=True,
                )
                nc.vector.reciprocal_approx_fast(recip_rep[:, j, :], den[:])

            def av_group(pr, j):
                # avT[dk, tokens] for head pair j, normalized by recip
                attn_m = attn_tiles[pr]
                recip_rep = recip_tiles[pr]
                v_sb = v_tiles[pr]
                if j == 0:
                    avT_tiles[pr] = attnp.tile(
                        [P, HC, PTOK], BF16, tag="avT", name="avT")
                avT = avT_tiles[pr]
                avp = ps_proj.tile([P, PTOK], F32, tag="p512")
                for h in (2 * j, 2 * j + 1):
                    p0 = 64 * (h % 2)
                    sl = slice(p0, p0 + 64)
                    tpos = None if p0 == 0 else (0, 64)
                    first = True
                    for r in range(2):
                        for c in range(NCH):
                            nc.tensor.matmul(
                                avp[sl, r * S + STRIPE_START[c]:
                                        r * S + STRIPE_START[c] + STRIPE],
                                v_sb[:, 2 * r + c, h * DK:(h + 1) * DK],
                                attn_m[:, h, c, r, :],
                                start=first,
                                stop=(r == 1 and c == NCH - 1),
                                tile_position=tpos,
                            )
                            first = False
                nc.vector.tensor_tensor(
                    avT[:, j, :], avp[:], recip_rep[:, j, :],
                    mybir.AluOpType.mult,
                )
                if j == HPAIRS - 1:
                    attn_tiles.pop(pr)
                    recip_tiles.pop(pr)
                    v_tiles.pop(pr)

            def stage_c(pr):
                avT = avT_tiles.pop(pr)
                o_sb = bpool.tile([P, TC, D], BF16, tag="o_sb")
                for tcc in range(TC):
                    pp = ps_proj.tile([P, PTOK], F32, tag="p512")
                    for hc in range(HC):
                        nc.tensor.matmul(
                            pp[:],
                            avT[:, hc, tcc * P:(tcc + 1) * P],
                            w2_sb[:, hc, :],
                            start=(hc == 0), stop=(hc == HC - 1),
                        )
                    if tcc % 2 == 0:
                        nc.scalar.activation(o_sb[:, tcc, :], pp[:], func=COPY)
                    else:
                        nc.vector.tensor_copy(o_sb[:, tcc, :], pp[:])
                    nc.sync.dma_start(
                        out_d[pr * PTOK + tcc * P:pr * PTOK + (tcc + 1) * P, :],
                        o_sb[:, tcc, :],
                    )

            # ---- software pipeline; scores/den/av interleaved per head so
            # PSUM-rotation waits are always covered by other PE work ----
            stage_load8(0)
            load_w1qk()
            stage_load(0)
            load_w1v()
            warmup_pe()
            if PAIRS > 1:
                stage_load8(1)
                stage_load(1)
            load_consts_rest()
            for t in range(PAIRS):
                stage_a1(t)
                if t + 2 < PAIRS:
                    stage_load8(t + 2)
                    stage_load(t + 2)
                for h in range(HEADS):
                    if 0 <= t - 1:
                        scores_head(t - 1, h)
                    if h < HPAIRS:
                        if 0 <= t - 2:
                            den_group(t - 2, h)
                    else:
                        if 0 <= t - 3:
                            av_group(t - 3, h - HPAIRS)
                if 0 <= t - 4:
                    stage_c(t - 4)

            # ---- eager drain: pair L's serial chain (exp->mask->den->recip->
            # av->mult->out) overlaps the leftover projections of pairs L-3..L-1
            L = PAIRS - 1
            for h in range(HEADS):
                scores_head(L, h, fast_mask=True)
                if h < HPAIRS:
                    den_group(L - 1, h)
                else:
                    av_group(L - 2, h - HPAIRS)
                if h % 2 == 1:
                    den_group(L, h // 2)
            stage_c(L - 3)
            for j in range(HPAIRS):
                av_group(L - 1, j)
            av_group(L, 0)
            av_group(L, 1)
            stage_c(L - 2)
            av_group(L, 2)
            av_group(L, 3)
            stage_c(L - 1)
            stage_c(L)

    nc.compile()
    return nc


def _in_maps(inputs, pos_bias, W_qkv, W_out):
    x = np.asarray(inputs, np.float32).astype(ml_dtypes.bfloat16)
    W1qk, W1v, W2, expB, ecE = _host_constants(
        np.asarray(pos_bias), np.asarray(W_qkv), np.asarray(W_out))
    x_flat = x.reshape(B * HW, S, D)
    in_maps = []
    for core in range(N_CORES):
        shard = x_flat[core * ROWS_PER_CORE:(core + 1) * ROWS_PER_CORE]
        # pre-transpose on host: xt[p, ko, t] = x[t, ko*128+p]
        xt = np.ascontiguousarray(
            shard.reshape(ROWS_PER_CORE * S, KO_G, P).transpose(2, 1, 0))
        u = np.zeros((2, P), ml_dtypes.bfloat16)
        u[0, :64] = 1.0
        u[1, 64:] = 1.0
        in_maps.append({
            "xt": xt, "x8": xt.astype(ml_dtypes.float8_e4m3),
            "w1qk": W1qk, "w1v": W1v, "w2": W2, "expb": expB, "ec": ecE,
            "u": u,
        })
    return in_maps


def kernel(inputs, pos_bias, W_qkv, W_out):
    if "nc" not in _CACHE:
        _CACHE["nc"] = _build_nc()
    nc = _CACHE["nc"]

    in_maps = _in_maps(inputs, pos_bias, W_qkv, W_out)
    res = run_bass_kernel_spmd(nc, in_maps, core_ids=list(range(N_CORES)))
    out = np.empty((B * HW, S, D), np.float32)
    for core in range(N_CORES):
        out[core * ROWS_PER_CORE:(core + 1) * ROWS_PER_CORE] = (
            res.results[core]["out"].astype(np.float32).reshape(
                ROWS_PER_CORE, S, D))
    return out.reshape(B, HW, S, D)

